# revision 8
# baseline (speedup 1.0000x reference)
"""Trainium2 Bass kernel for nn_Attention_8495445311883.

Encoder (bi-RNN) + decoder + dot-attention + output projection.
Sharding: data-parallel over batch B=32 across 8 NeuronCores (4 batches/core).
All matmuls in bf16 (fp32 PSUM accumulate). Host pre-packs/transposes weights.

Per-core column index c = b_local*T + t  (b-major), C = 4*T = 512.

Schedule (single pass, PE-dense):
  pass A (enc input proj, 8 psum banks, 100% PE)
  -> scan region: per-step chain PE(idmm+32 recurrence mm)->ACT(tanh, reads
     PSUM) with pass-B matmuls interleaved one-at-a-time as PE filler
  -> mix: h0/q chain, ENC proj, DEC tanh, ENCT via PE transpose, attention
     (software-pipelined over the 4 local batches)
  -> final projection (20 V-chunks of 500, bias added once per chunk via DVE)
"""
import os
import sys
import numpy as np

sys.path.insert(0, "/opt/trn_rl_repo")

V, H, T, B = 10000, 512, 128, 32
NCORES = 8
BL = B // NCORES            # 4 local batches
C = BL * T                  # 512 columns per core
VP = 10112                  # V padded to 79*128
KV = VP // 128              # 79 contraction tiles
NV, VC = 20, 500            # output V chunks: 20 x 500
KG = 4                      # k-tiles per DMA chunk
NKG = (KV + KG - 1) // KG   # 20 chunks, last has 3

_cached = {}


def _build_nc(reps=1):
    import concourse.bacc as bacc
    import concourse.mybir as mybir
    import concourse.tile as tile

    dt = mybir.dt
    AF = mybir.ActivationFunctionType
    AX = mybir.AxisListType

    nc = bacc.Bacc(None, target_bir_lowering=False)

    xT = nc.dram_tensor("xT", [VP, C], dt.bfloat16, kind="ExternalInput")
    dxT = nc.dram_tensor("dxT", [VP, C], dt.bfloat16, kind="ExternalInput")
    WIH = nc.dram_tensor("WIH", [VP, 3 * H], dt.bfloat16, kind="ExternalInput")
    WO = nc.dram_tensor("WO", [2 * H, V], dt.bfloat16, kind="ExternalInput")
    WHH = nc.dram_tensor("WHH", [H, 3 * H], dt.bfloat16, kind="ExternalInput")
    A1 = nc.dram_tensor("A1", [2 * H, H], dt.bfloat16, kind="ExternalInput")
    A2 = nc.dram_tensor("A2", [2 * H, H], dt.bfloat16, kind="ExternalInput")
    CONST = nc.dram_tensor("CONST", [128, 12], dt.float32, kind="ExternalInput")
    BOUT = nc.dram_tensor("BOUT", [1, V], dt.bfloat16, kind="ExternalInput")
    ONES = nc.dram_tensor("ONES", [1, 128], dt.bfloat16, kind="ExternalInput")
    IDN = nc.dram_tensor("IDN", [128, 128], dt.bfloat16, kind="ExternalInput")
    IDN16 = nc.dram_tensor("IDN16", [128, 128], dt.float16, kind="ExternalInput")
    ENCH = nc.dram_tensor("ENCH", [128, 32], dt.bfloat16, kind="ExternalInput")
    out = nc.dram_tensor("out", [BL, T, V], dt.float32, kind="ExternalOutput")

    xTr = xT.rearrange("(k p) c -> p k c", p=128)
    dxTr = dxT.rearrange("(k p) c -> p k c", p=128)
    WIr = WIH.rearrange("(k p) c -> p k c", p=128)
    WOr = WO.rearrange("(k p) v -> p k v", p=128)
    outr = out.rearrange("b t v -> t b v")

    with tile.TileContext(nc) as tc:
        with (
            tc.tile_pool(name="const", bufs=1) as cp,
            tc.tile_pool(name="acts", bufs=1) as ap,
            tc.tile_pool(name="xs", bufs=3) as xs,
            tc.tile_pool(name="ws", bufs=2) as ws,
            tc.tile_pool(name="os", bufs=4) as osp,
        ):
            # ---- persistent activations ----
            PREF = ap.tile([128, 4, C], dt.float16, tag="PREF")
            PREB = ap.tile([128, 4, C], dt.float16, tag="PREB")
            PRED = ap.tile([128, 4, C], dt.float16, tag="PRED")
            OUTF = ap.tile([128, 4, C], dt.bfloat16, tag="OUTF")
            OUTB = ap.tile([128, 4, C], dt.bfloat16, tag="OUTB")
            ENC = ap.tile([128, 4, C], dt.bfloat16, tag="ENC")
            ENCT = ap.tile([128, 4, C], dt.bfloat16, tag="ENCT")
            DEC = ap.tile([128, 4, C], dt.bfloat16, tag="DEC")
            CTX = ap.tile([128, 4, C], dt.bfloat16, tag="CTX")
            H0 = ap.tile([128, 4, 4], dt.bfloat16, tag="H0")
            Q = ap.tile([128, 4, 4], dt.float32, tag="Q")

            # ---- resident constants/weights (DVE queue; after first chunks) ----
            def load_consts():
                whh = cp.tile([128, 4, 3 * H], dt.bfloat16, tag="whh")
                nc.gpsimd.dma_start(whh[:], WHH.rearrange("(j p) c -> p j c", p=128))
                a1 = cp.tile([128, 8, H], dt.bfloat16, tag="a1")
                nc.gpsimd.dma_start(a1[:], A1.rearrange("(j p) c -> p j c", p=128))
                a2 = cp.tile([128, 8, H], dt.bfloat16, tag="a2")
                nc.gpsimd.dma_start(a2[:], A2.rearrange("(j p) c -> p j c", p=128))
                cst = cp.tile([128, 12], dt.float32, tag="cst")
                nc.gpsimd.dma_start(cst[:], CONST[:])
                bout = cp.tile([1, V], dt.bfloat16, tag="bout")
                nc.gpsimd.dma_start(bout[:], BOUT[:])
                ones = cp.tile([1, 128], dt.bfloat16, tag="ones")
                nc.gpsimd.dma_start(ones[:], ONES[:])
                idn = cp.tile([128, 128], dt.bfloat16, tag="idn")
                nc.gpsimd.dma_start(idn[:], IDN[:])
                idn16 = cp.tile([128, 128], dt.float16, tag="idn16")
                nc.gpsimd.dma_start(idn16[:], IDN16[:])
                ench = cp.tile([128, 32], dt.bfloat16, tag="ench")
                nc.gpsimd.dma_start(ench[:], ENCH[:])
                return whh, a1, a2, cst, bout, ones, idn, idn16, ench

            consts = None

            for _rep in range(reps):
                # ---- pass A: encoder input projection (f+b) ----
                pA_cm = tc.tile_pool(name="pA", bufs=1, space="PSUM")
                pA = pA_cm.__enter__()
                psa = [pA.tile([128, C], dt.float32, tag=f"a{m}", name=f"psa{m}")
                       for m in range(8)]

                # chunk DMAs: xk on SP queue, wk on ACT queue (idle in pass A);
                # pool rotation self-paces the streams.
                a_tiles = []
                for g in range(NKG):
                    ks = (g * KG, min((g + 1) * KG, KV))
                    nk = ks[1] - ks[0]
                    xk = xs.tile([128, KG, C], dt.bfloat16, tag="xk")
                    nc.sync.dma_start(xk[:, :nk, :], xTr[:, ks[0]:ks[1], :])
                    wk = ws.tile([128, KG, 2 * H], dt.bfloat16, tag="wk")
                    nc.scalar.dma_start(wk[:, :nk, :], WIr[:, ks[0]:ks[1], 0:2 * H])
                    a_tiles.append((xk, wk, ks))
                    if g == 5 and consts is None:
                        consts = load_consts()
                whh, a1, a2, cst, bout, ones, idn, idn16, ench = consts

                for xk, wk, ks in a_tiles[:-1]:
                    for i in range(ks[1] - ks[0]):
                        k = ks[0] + i
                        for m in range(8):
                            nc.tensor.matmul(
                                psa[m][:], wk[:, i, m * 128:(m + 1) * 128],
                                xk[:, i, :],
                                start=(k == 0), stop=False,
                            )
                # last chunk m-outer so each PSUM->fp16 pre copy (ACT) overlaps
                # the remaining m-groups' matmuls
                xk, wk, ks = a_tiles[-1]
                for m in range(8):
                    for i in range(ks[1] - ks[0]):
                        k = ks[0] + i
                        nc.tensor.matmul(
                            psa[m][:], wk[:, i, m * 128:(m + 1) * 128],
                            xk[:, i, :],
                            start=False, stop=(k == KV - 1),
                        )
                    dst = PREF if m < 4 else PREB
                    nc.scalar.activation(dst[:, m % 4, :], psa[m][:], AF.Copy)
                pA_cm.__exit__(None, None, None)

                # ---- scan region: fwd+bwd RNN with pass-B matmuls as filler ----
                pB_cm = tc.tile_pool(name="pB", bufs=1, space="PSUM")
                pB = pB_cm.__enter__()
                psc_cm = tc.tile_pool(name="psc", bufs=1, space="PSUM")
                psc = psc_cm.__enter__()
                psb = [pB.tile([128, C], dt.float32, tag=f"b{m}", name=f"psb{m}")
                       for m in range(4)]

                # pre-issue all pass-B chunk DMAs (self-pacing via bufs)
                b_tiles = []
                for g in range(NKG):
                    ks = (g * KG, min((g + 1) * KG, KV))
                    nk = ks[1] - ks[0]
                    dk = xs.tile([128, KG, C], dt.bfloat16, tag="dk", bufs=3)
                    nc.gpsimd.dma_start(dk[:, :nk, :], dxTr[:, ks[0]:ks[1], :])
                    wkd = ws.tile([128, KG, H], dt.bfloat16, tag="wkd", bufs=3)
                    nc.gpsimd.dma_start(wkd[:, :nk, :], WIr[:, ks[0]:ks[1], 2 * H:])
                    b_tiles.append((dk, wkd, ks))
                # prefetch first two final-proj weight chunks (gpsimd queue,
                # behind the wkd stream; arrives during scan/mix)
                won_tiles = []
                for n in range(NV):
                    won = ws.tile([128, 8, VC], dt.bfloat16, tag="won")
                    nc.gpsimd.dma_start(won[:], WOr[:, :, n * VC:(n + 1) * VC])
                    won_tiles.append(won)

                def b_mm_gen():
                    for dk, wkd, ks in b_tiles:
                        for i in range(ks[1] - ks[0]):
                            k = ks[0] + i
                            for m in range(4):
                                nc.tensor.matmul(
                                    psb[m][:], wkd[:, i, m * 128:(m + 1) * 128],
                                    dk[:, i, :],
                                    start=(k == 0), stop=(k == KV - 1),
                                )
                                yield

                bgen = b_mm_gen()
                bdone = [0]
                TOTB = KV * 4

                def pump_b(target):
                    while bdone[0] < min(target, TOTB):
                        try:
                            next(bgen)
                        except StopIteration:
                            bdone[0] = TOTB
                            return
                        bdone[0] += 1

                for t in range(T):
                    tb = T - 1 - t
                    scf = psc.tile([128, 4, 4], dt.float32, tag="scf", bufs=2)
                    nc.tensor.matmul(scf[:], idn16[:], PREF[:, :, t::T],
                                     start=True, stop=False)
                    for m in range(4):
                        for j in range(4):
                            rf = ench[:, j * 4:(j + 1) * 4] if t == 0 else \
                                OUTF[:, j, (t - 1)::T]
                            nc.tensor.matmul(
                                scf[:, m, :], whh[:, j, m * 128:(m + 1) * 128], rf,
                                start=False, stop=(m == 3 and j == 3),
                            )
                    nc.scalar.activation(OUTF[:, :, t::T], scf[:], AF.Tanh)
                    scb = psc.tile([128, 4, 4], dt.float32, tag="scb", bufs=2)
                    nc.tensor.matmul(scb[:], idn16[:], PREB[:, :, tb::T],
                                     start=True, stop=False)
                    for m in range(4):
                        for j in range(4):
                            rb = ench[:, 16 + j * 4:16 + (j + 1) * 4] if t == 0 \
                                else OUTB[:, j, (tb + 1)::T]
                            nc.tensor.matmul(
                                scb[:, m, :], whh[:, j, H + m * 128:H + (m + 1) * 128],
                                rb, start=False, stop=(m == 3 and j == 3),
                            )
                    nc.scalar.activation(OUTB[:, :, tb::T], scb[:], AF.Tanh)
                    # pass-B filler, paced to finish by step ~120
                    pump_b((t + 1) * TOTB // 120 + 1)
                pump_b(TOTB)
                # PSUM -> PRED fp16 (DVE; ACT still busy with scan tail)
                for m in range(4):
                    nc.vector.tensor_copy(PRED[:, m, :], psb[m][:])
                psc_cm.__exit__(None, None, None)
                pB_cm.__exit__(None, None, None)

                # ---- mix phase ----
                pmix_cm = tc.tile_pool(name="pmix", bufs=1, space="PSUM")
                pmix = pmix_cm.__enter__()

                # h0 = A1 @ [h_f; h_b] + b_attn1
                ph = pmix.tile([128, 4, 4], dt.float32, tag="pscal")
                for m in range(4):
                    for k in range(8):
                        rh = OUTF[:, k, (T - 1)::T] if k < 4 else OUTB[:, k - 4, 0::T]
                        nc.tensor.matmul(ph[:, m, :], a1[:, k, m * 128:(m + 1) * 128],
                                         rh, start=(k == 0), stop=(k == 7))
                for m in range(4):
                    nc.scalar.activation(H0[:, m, :], ph[:, m, :], AF.Identity,
                                         bias=cst[:, m:m + 1])
                # q = Whh_d @ h0 + bhh_d
                pq = pmix.tile([128, 4, 4], dt.float32, tag="pscal")
                for m in range(4):
                    for j in range(4):
                        nc.tensor.matmul(
                            pq[:, m, :], whh[:, j, 2 * H + m * 128:2 * H + (m + 1) * 128],
                            H0[:, j, :], start=(j == 0), stop=(j == 3),
                        )
                for m in range(4):
                    nc.scalar.activation(Q[:, m, :], pq[:, m, :], AF.Identity,
                                         bias=cst[:, 8 + m:9 + m])

                # ENC = W_attn2 @ [out_f; out_b] + b_attn2, with DEC tanh
                # interleaved on ACT and ENCT transposes pipelined on PE
                def enc_mms(m):
                    pe = pmix.tile([128, C], dt.float32, tag="pe2", bufs=2)
                    for k in range(8):
                        src = OUTF if k < 4 else OUTB
                        nc.tensor.matmul(
                            pe[:], a2[:, k, m * 128:(m + 1) * 128], src[:, k % 4, :],
                            start=(k == 0), stop=(k == 7),
                        )
                    return pe

                def enct_tr(m):
                    for b in range(BL):
                        ptr = pmix.tile([128, 128], dt.bfloat16, tag="ptb", bufs=2)
                        nc.tensor.transpose(ptr[:], ENC[:, m, b * T:(b + 1) * T],
                                            idn[:])
                        nc.vector.tensor_copy(
                            ENCT[:, b, m * 128:(m + 1) * 128], ptr[:])

                pe_tiles = {}
                for m in range(4):
                    pe_tiles[m] = enc_mms(m)
                    nc.scalar.activation(ENC[:, m, :], pe_tiles[m][:], AF.Identity,
                                         bias=cst[:, 4 + m:5 + m])
                    # DEC tanh for batch m index (spread over ACT)
                    for b in range(BL):
                        nc.scalar.activation(
                            DEC[:, m, b * T:(b + 1) * T], PRED[:, m, b * T:(b + 1) * T],
                            AF.Tanh, bias=Q[:, m, b:b + 1],
                        )
                    if m > 0:
                        enct_tr(m - 1)
                enct_tr(3)

                # ---- attention, software-pipelined over batches ----
                def scores_chain(b):
                    ps = pmix.tile([128, 128], dt.float32, tag="pf128", bufs=2)
                    for k in range(4):
                        nc.tensor.matmul(
                            ps[:], DEC[:, k, b * T:(b + 1) * T],
                            ENC[:, k, b * T:(b + 1) * T],
                            start=(k == 0), stop=(k == 3),
                        )
                    negm = osp.tile([128, 1], dt.float32, tag="negm")
                    nc.vector.reduce_max(negm[:], ps[:], axis=AX.X, negate=True)
                    prob = osp.tile([128, T], dt.bfloat16, tag="prob")
                    rsum = osp.tile([128, 1], dt.float32, tag="rsum")
                    nc.scalar.activation(prob[:], ps[:], AF.Exp, bias=negm[:],
                                         accum_out=rsum[:])
                    rinv = osp.tile([128, 1], dt.float32, tag="rinv")
                    nc.vector.reciprocal(rinv[:], rsum[:])
                    nc.vector.tensor_scalar_mul(prob[:], prob[:], rinv[:])
                    return prob

                def ctx_chain(b, prob):
                    pwt = pmix.tile([128, 128], dt.bfloat16, tag="ptb", bufs=2)
                    nc.tensor.transpose(pwt[:], prob[:], idn[:])
                    wt = osp.tile([128, T], dt.bfloat16, tag="wt")
                    nc.vector.tensor_copy(wt[:], pwt[:])
                    for m in range(4):
                        pc = pmix.tile([128, 128], dt.float32, tag="pf128", bufs=2)
                        nc.tensor.matmul(pc[:], ENCT[:, b, m * 128:(m + 1) * 128],
                                         wt[:], start=True, stop=True)
                        nc.vector.tensor_copy(CTX[:, m, b * T:(b + 1) * T], pc[:])

                probs = {}
                for b in range(BL):
                    probs[b] = scores_chain(b)
                    if b > 0:
                        ctx_chain(b - 1, probs[b - 1])
                ctx_chain(3, probs[3])
                pmix_cm.__exit__(None, None, None)

                # ---- final projection: predict[c, v] ----
                pf_cm = tc.tile_pool(name="pf", bufs=1, space="PSUM")
                pf = pf_cm.__enter__()
                for n in range(NV):
                    won = won_tiles[n]
                    pbias = pf.tile([128, VC], dt.float32, tag="pbias", bufs=2)
                    nc.tensor.matmul(pbias[:], ones[0:1, :],
                                     bout[0:1, n * VC:(n + 1) * VC],
                                     start=True, stop=True)
                    bias_sb = osp.tile([128, VC], dt.bfloat16, tag="bsb", bufs=2)
                    nc.scalar.activation(bias_sb[:], pbias[:], AF.Copy)
                    ob = osp.tile([128, BL, VC], dt.float32, tag="ob", bufs=2)
                    for b in range(BL):
                        po = pf.tile([128, VC], dt.float32, tag="po", bufs=4)
                        for k in range(8):
                            src = CTX if k < 4 else DEC
                            nc.tensor.matmul(
                                po[:], src[:, k % 4, b * T:(b + 1) * T],
                                won[:, k, :],
                                start=(k == 0), stop=(k == 7),
                            )
                        nc.vector.tensor_add(ob[:, b, :], po[:], bias_sb[:])
                    nc.sync.dma_start(outr[:, :, n * VC:(n + 1) * VC], ob[:])
                pf_cm.__exit__(None, None, None)

    nc.compile()
    return nc


def _pack(inputs):
    """Host-side packing: shared weights + per-core activation shards."""
    import ml_dtypes
    bf16 = ml_dtypes.bfloat16
    f = {k: np.asarray(v, dtype=np.float32) for k, v in inputs.items()}

    def bf(a):
        return np.ascontiguousarray(a.astype(bf16))

    WIH = np.zeros((VP, 3 * H), np.float32)
    WIH[:V, 0:H] = f["Wih_f"].T
    WIH[:V, H:2 * H] = f["Wih_b"].T
    WIH[:V, 2 * H:] = f["Wih_d"].T
    WIH[V, 0:H] = f["bih_f"] + f["bhh_f"]
    WIH[V, H:2 * H] = f["bih_b"] + f["bhh_b"]
    WIH[V, 2 * H:] = f["bih_d"]

    WHH = np.concatenate([f["Whh_f"].T, f["Whh_b"].T, f["Whh_d"].T], axis=1)
    CONST = np.concatenate(
        [f["b_attn1"].reshape(4, 128).T, f["b_attn2"].reshape(4, 128).T,
         f["bhh_d"].reshape(4, 128).T], axis=1).astype(np.float32)

    shared = {
        "WIH": bf(WIH),
        "WO": bf(f["W_out"].T),
        "WHH": bf(WHH),
        "A1": bf(f["W_attn1"].T),
        "A2": bf(f["W_attn2"].T),
        "CONST": np.ascontiguousarray(CONST),
        "BOUT": bf(f["b_out"].reshape(1, V)),
        "ONES": bf(np.ones((1, 128), np.float32)),
        "IDN": bf(np.eye(128, dtype=np.float32)),
        "IDN16": np.ascontiguousarray(np.eye(128, dtype=np.float16)),
    }

    # activations: [V, B*T] with column b*T + t; pad to VP with ones row at V
    def actT(a):  # [B, T, V] -> [VP, B*T]
        r = np.zeros((VP, B * T), np.float32)
        r[:V] = a.transpose(2, 0, 1).reshape(V, B * T)
        r[V] = 1.0
        return bf(r)

    XT = actT(f["enc_inputs"])
    DXT = actT(f["dec_inputs"])

    in_maps = []
    for core in range(NCORES):
        sl = slice(core * C, (core + 1) * C)
        eh = np.zeros((128, 32), np.float32)
        for d in range(2):
            hh = f["enc_hidden"][d, core * BL:(core + 1) * BL]     # [4, 512]
            eh[:, d * 16:(d + 1) * 16] = \
                hh.T.reshape(4, 128, 4).transpose(1, 0, 2).reshape(128, 16)
        m = dict(shared)
        m["xT"] = np.ascontiguousarray(XT[:, sl])
        m["dxT"] = np.ascontiguousarray(DXT[:, sl])
        m["ENCH"] = bf(eh)
        in_maps.append(m)
    return in_maps


def kernel(**inputs):
    from concourse.bass_utils import run_bass_kernel_spmd

    if "nc" not in _cached:
        _cached["nc"] = _build_nc()
    nc = _cached["nc"]
    in_maps = _pack(inputs)
    res = run_bass_kernel_spmd(
        nc, in_maps, core_ids=list(range(NCORES)),
        trace=bool(int(os.environ.get("KTRACE", "0"))),
    )
    _cached["last"] = res
    outp = np.zeros((B, T, V), np.float32)
    for core in range(NCORES):
        outp[core * BL:(core + 1) * BL] = res.results[core]["out"]
    return outp
